# revision 15
# baseline (speedup 1.0000x reference)
"""NVFP4-fake-quant MLP (x@w1.T -> gelu -> @w2.T) on 8 trn2 NeuronCores.

Sharding (megatron tensor-parallel on the hidden dim):
  core c holds w1 rows [c*2048:(c+1)*2048], w2 cols [c*2048:(c+1)*2048],
  and x rows [c*1024:(c+1)*1024] (for distributed x-quantization).

Quantization (bit-identical to the magic-round reference decomposition):
  per-16-block e4m3 scales via exponent-mask + magic-number RNE;
  fp4 e2m1 rounding via a single exponent-extract magic round:
    s  = clamp(exp2bits(r), 1, 4)         (binade -> grid selector)
    M  = clamp(s * 1.5*2^22, C, 4C)       (magic const, grid = s/2)
    q  = (r + M) - M                      (RNE on the e2m1 grid, signed)
  e2m1_value * e4m3_blockscale has <= 6 mantissa bits -> stored EXACTLY in
  bf16, so the bf16 matmuls reproduce the f32 reference; per-tensor scales
  are folded into the PSUM->SBUF copies (gelu input scale / output scale).

Dataflow / overlap:
  x and w1 stream concurrently; x is quantized in 4 column chunks, each
  transposed and AllGathered immediately (xqT stored b-tile-contiguous so
  phase-1 lhsT loads are single wide DMAs).  w1-amax -> AllReduce -> w1
  quant in 4 column chunks with w1T transposes chasing, so phase-1 matmuls
  ramp as chunks land.  w2 amax/AllReduce/quant overlap phase 1.  Phase-2
  partials are stored bf16 and ReduceScattered bf16 in 8 chunks overlapped
  with compute; rsout -> f32 conversion is done on-chip at the tail.
"""
import os
import sys
import numpy as np

if "/opt/trn_rl_repo" not in sys.path:
    sys.path.insert(0, "/opt/trn_rl_repo")

f32 = np.float32

B, D_IN, HID, D_OUT = 8192, 4096, 16384, 4096
NCORES = 8
BSH = B // NCORES          # 1024 x-rows quantized per core
HSH = HID // NCORES        # 2048 hidden units per core
NK1 = D_IN // 128          # 32 k-tiles, first matmul
NK2 = HSH // 128           # 16 k-tiles, second matmul
NBT = B // 128             # 64 b-tiles
KC = 4                     # x / w1 column chunks (1024 cols = 8 k-tiles)
SBR = 512                  # phase-2 superblock rows
NSB = B // SBR             # 16 superblocks
RSCH = 8                   # reduce-scatter chunks (= 2 superblocks each)
RSROWS = B // RSCH         # 1024 rows per RS chunk

# magic round-to-nearest-even constants (f32-exact)
C_E2M1 = float(f32(1.5 * 2 ** 22))       # x 2^e -> magic for grid 2^(e-1)
E4M3_MAGIC = float(f32(1.5 * 2 ** 20))   # x 2^e -> magic const for step 2^(e-3)
EXPMASK = 0x7F800000

_BUILT = {}


def _emit_quant(nc, mybir, pf, pb, pn, src, out, c1, effmul, W, eng):
    """Quantize src [128, W] f32 (SBUF) -> out [128, W] bf16 = e2m1*bscale.

    c1: 1/(6*tensor_scale)  (float imm or [128,1] AP)
    effmul: tensor_scale    (float imm or [128,1] AP)
    eng: engine for the full-width elementwise chain (nc.vector/nc.gpsimd);
         block-width ops and the bitwise exponent extract stay on vector.
    """
    OP = mybir.AluOpType
    U32 = mybir.dt.uint32
    FP32 = mybir.dt.float32
    BF16 = mybir.dt.bfloat16
    NB = W // 16

    amax = pn.tile([128, NB], FP32, tag="q_amax", name="q_amax")
    nc.vector.tensor_reduce(
        amax[:], src.rearrange("p (nb b) -> p nb b", b=16),
        axis=mybir.AxisListType.X, op=OP.max, apply_absolute_value=True)
    vq = pn.tile([128, NB], FP32, tag="q_vq", name="q_vq")
    nc.vector.tensor_scalar(vq[:], amax[:], c1, None, OP.mult)
    scq = pn.tile([128, NB], FP32, tag="q_scq", name="q_scq")
    nc.vector.tensor_scalar(scq[:].bitcast(U32), vq[:].bitcast(U32),
                            EXPMASK, None, OP.bitwise_and)
    cb = pn.tile([128, NB], FP32, tag="q_cb", name="q_cb")
    nc.vector.tensor_scalar(cb[:], scq[:], E4M3_MAGIC, None, OP.mult)
    t4 = pn.tile([128, NB], FP32, tag="q_t4", name="q_t4")
    nc.vector.tensor_tensor(t4[:], vq[:], cb[:], OP.add)
    bs = pn.tile([128, NB], FP32, tag="q_bs", name="q_bs")
    nc.vector.tensor_tensor(bs[:], t4[:], cb[:], OP.subtract)
    bs16 = pn.tile([128, NB], BF16, tag="q_bs16", name="q_bs16")
    nc.vector.tensor_scalar(bs16[:], bs[:], 2.0 ** -6, None, OP.max)
    eff = pn.tile([128, NB], FP32, tag="q_eff", name="q_eff")
    nc.vector.tensor_scalar(eff[:], bs[:], 2.0 ** -6, effmul, OP.max, OP.mult)
    rec = pn.tile([128, NB], FP32, tag="q_rec", name="q_rec")
    nc.vector.reciprocal(rec[:], eff[:])

    r = pf.tile([128, W], FP32, tag="q_r", name="q_r")
    eng.tensor_tensor(
        r[:].rearrange("p (nb b) -> p nb b", b=16),
        src.rearrange("p (nb b) -> p nb b", b=16),
        rec[:, :, None].to_broadcast([128, NB, 16]), OP.mult)
    sx = pf.tile([128, W], FP32, tag="q_sx", name="q_sx")
    nc.vector.tensor_scalar(sx[:].bitcast(U32), r[:].bitcast(U32),
                            EXPMASK, None, OP.bitwise_and)
    sM = pf.tile([128, W], FP32, tag="q_sM", name="q_sM")
    eng.tensor_scalar(sM[:], sx[:], C_E2M1, C_E2M1, OP.mult, OP.max)
    eng.tensor_scalar(sM[:], sM[:], 4.0 * C_E2M1, None, OP.min)
    eng.tensor_tensor(r[:], r[:], sM[:], OP.add)       # r -> r + M
    eng.tensor_tensor(r[:], r[:], sM[:], OP.subtract)  # -> q (RNE on grid)
    eng.tensor_tensor(
        out.rearrange("p (nb b) -> p nb b", b=16),
        r[:].rearrange("p (nb b) -> p nb b", b=16),
        bs16[:, :, None].to_broadcast([128, NB, 16]), OP.mult)


def _build(isc, hsc):
    from contextlib import ExitStack
    import concourse.bass as bass
    import concourse.tile as tile
    from concourse import bacc, mybir

    OP = mybir.AluOpType
    AF = mybir.ActivationFunctionType
    FP32 = mybir.dt.float32
    BF16 = mybir.dt.bfloat16

    c1x = float(f32(1.0) / (f32(6.0) * f32(isc)))
    c1h = float(f32(1.0) / (f32(6.0) * f32(hsc)))
    inv2688 = float(f32(1.0) / f32(2688.0))
    RG = [list(range(NCORES))]

    nc = bacc.Bacc("TRN2", target_bir_lowering=False, debug=False,
                   num_devices=NCORES)
    x_sh = nc.dram_tensor("x_sh", [BSH, D_IN], FP32, kind="ExternalInput").ap()
    w1_sh = nc.dram_tensor("w1_sh", [HSH, D_IN], FP32, kind="ExternalInput").ap()
    w2_sh = nc.dram_tensor("w2_sh", [D_OUT, HSH], FP32, kind="ExternalInput").ap()
    out_sh = nc.dram_tensor("out_sh", [BSH, D_OUT], FP32, kind="ExternalOutput").ap()
    debug = os.environ.get("KQ_DEBUG", "0") == "1"
    if debug:
        dbg_xq = nc.dram_tensor("dbg_xq", [BSH, D_IN], BF16,
                                kind="ExternalOutput").ap()
        dbg_w1q = nc.dram_tensor("dbg_w1q", [HSH, D_IN], BF16,
                                 kind="ExternalOutput").ap()
        dbg_w2q = nc.dram_tensor("dbg_w2q", [D_OUT, HSH], BF16,
                                 kind="ExternalOutput").ap()
        dbg_hq = nc.dram_tensor("dbg_hq", [B, HSH], BF16,
                                kind="ExternalOutput").ap()
        dbg_xf0 = nc.dram_tensor("dbg_xf0", [NCORES * BSH, 1024], BF16,
                                 kind="ExternalOutput").ap()
        dbg_part0 = nc.dram_tensor("dbg_part0", [RSROWS, D_OUT], BF16,
                                   kind="ExternalOutput").ap()
        dbg_rsout0 = nc.dram_tensor("dbg_rsout0", [128, D_OUT], BF16,
                                    kind="ExternalOutput").ap()
        dbg_w2T = nc.dram_tensor("dbg_w2T", [128, NK2 * D_OUT], BF16,
                                 kind="ExternalOutput").ap()
        dbg_hT0 = nc.dram_tensor("dbg_hT0", [128, NK2 * SBR], BF16,
                                 kind="ExternalOutput").ap()

    with tile.TileContext(nc) as tc, ExitStack() as top:
        dram = top.enter_context(tc.tile_pool(name="dram", bufs=1, space="DRAM"))
        amax_stage = dram.tile([128, 2], FP32, tag="amax_stage", name="amax_stage")
        s1loc = dram.tile([1, 1], FP32, tag="s1loc", name="s1loc")
        s2loc = dram.tile([1, 1], FP32, tag="s2loc", name="s2loc")
        s1sh = dram.tile([1, 1], FP32, tag="s1sh", name="s1sh", addr_space="Shared")
        s2sh = dram.tile([1, 1], FP32, tag="s2sh", name="s2sh", addr_space="Shared")
        xq_loc = dram.tile([BSH, D_IN], BF16, tag="xq_loc", name="xq_loc")
        # per-chunk xqT, laid out so phase-1 lhsT loads are contiguous:
        # [bt(8) * p(128), kt(8) * b(128)]
        xqT_cs = [dram.tile([BSH, 1024], BF16, name=f"xqT_c{c}", tag=f"xqT_c{c}")
                  for c in range(KC)]
        xqT_fs = [dram.tile([NCORES * BSH, 1024], BF16, name=f"xqT_f{c}",
                            tag=f"xqT_f{c}", addr_space="Shared")
                  for c in range(KC)]
        w1q = dram.tile([HSH, D_IN], BF16, tag="w1q", name="w1q")
        w2q = dram.tile([D_OUT, HSH], BF16, tag="w2q", name="w2q")
        hq = dram.tile([B, HSH], BF16, tag="hq", name="hq")
        parts = [dram.tile([RSROWS, D_OUT], BF16, name=f"part{c}", tag=f"part{c}")
                 for c in range(RSCH)]
        rsouts = [dram.tile([128, D_OUT], BF16, name=f"rsout{c}", tag=f"rsout{c}")
                  for c in range(RSCH)]

        singles = top.enter_context(tc.tile_pool(name="singles", bufs=1))

        # w1T pool pre-reserved below the phase-0/1 scratch so its transpose
        # loads can chase w1-quant without a pool-region handoff.
        w1T_cm = tc.tile_pool(name="w1T", bufs=1)
        w1T_pool = w1T_cm.__enter__()
        w1T = w1T_pool.tile([128, NK1, HSH], BF16, tag="w1T", name="w1T")

        # ============ Phases 0 + 1 (shared pools) ============
        with tc.tile_pool(name="p0src", bufs=2) as p0src, \
             tc.tile_pool(name="pf", bufs=2) as pf, \
             tc.tile_pool(name="pb", bufs=3) as pb, \
             tc.tile_pool(name="pn", bufs=2) as pn, \
             tc.tile_pool(name="xtt", bufs=2) as xtt_pool, \
             tc.tile_pool(name="xb", bufs=2) as xb_pool, \
             tc.tile_pool(name="ps1", bufs=8, space="PSUM") as ps1:

            acc1 = singles.tile([128, 1], FP32, tag="acc1", name="acc1")
            nquant = [0]

            nogps = os.environ.get("KQ_NOGPS", "0") == "1"

            def qeng():
                # ~1 of 6 full-width chains go to gpsimd (it is ~4x slower)
                nquant[0] += 1
                return nc.gpsimd if (nquant[0] % 6 == 0 and not nogps) \
                    else nc.vector

            # ---- interleaved x-quant (chunked, feeding AGs) + w1 amax ----
            namax = [0]
            for c in range(KC):
                csl = slice(c * 1024, (c + 1) * 1024)
                # x chunk c: load + quantize + store xq
                for i in range(BSH // 128):
                    xt = p0src.tile([128, 1024], FP32, tag="src", name="xt")
                    nc.scalar.dma_start(xt[:], x_sh[i * 128:(i + 1) * 128, csl])
                    xo = pb.tile([128, 1024], BF16, tag="qout", name="xo")
                    _emit_quant(nc, mybir, pf, pb, pn, xt[:], xo[:],
                                c1x, float(isc), 1024, qeng())
                    nc.sync.dma_start(xq_loc[i * 128:(i + 1) * 128, csl], xo[:])
                # w1 amax stream for this chunk (16 row-tiles x 1024 cols)
                for j in range(HSH // 128):
                    wt = p0src.tile([128, 1024], FP32, tag="wsrc", name="wamax",
                                    bufs=2)
                    nc.scalar.dma_start(
                        wt[:], w1_sh[j * 128:(j + 1) * 128, csl])
                    am = pn.tile([128, 1], FP32, tag="am_w", name="am_w")
                    nc.vector.tensor_reduce(am[:], wt[:],
                                            axis=mybir.AxisListType.X,
                                            op=OP.max, apply_absolute_value=True)
                    if namax[0] == 0:
                        nc.vector.tensor_copy(acc1[:], am[:])
                    else:
                        nc.vector.tensor_tensor(acc1[:], acc1[:], am[:], OP.max)
                    namax[0] += 1
                # transpose chunk c into b-tile-contiguous layout + AllGather
                xqT_v = xqT_cs[c][:].rearrange(
                    "(bt p) (k b) -> bt p k b", p=128, b=128)
                for j in range(8):
                    kt = c * 8 + j
                    xt2 = xtt_pool.tile([128, BSH], BF16, tag="xtt", name="xtt")
                    nc.sync.dma_start(xt2[:], xq_loc[:, kt * 128:(kt + 1) * 128],
                                      transpose=True)
                    for bt in range(8):
                        nc.sync.dma_start(
                            xqT_v[bt, :, j, :],
                            xt2[:, bt * 128:(bt + 1) * 128])
                nc.gpsimd.collective_compute(
                    "AllGather", OP.bypass, replica_groups=RG,
                    ins=[xqT_cs[c][:].opt()], outs=[xqT_fs[c][:].opt()])

            # ---- w1 scale: local reduce -> AllReduce(max) -> scalars ----
            nc.sync.dma_start(amax_stage[:, 0:1], acc1[:])
            rowv1 = singles.tile([1, 128], FP32, tag="rowv1", name="rowv1")
            nc.sync.dma_start(
                rowv1[:], amax_stage[:, 0:1].rearrange("p c -> (p c)").unsqueeze(0))
            red1 = singles.tile([1, 1], FP32, tag="red1", name="red1")
            nc.vector.tensor_reduce(red1[:], rowv1[:],
                                    axis=mybir.AxisListType.X, op=OP.max)
            nc.sync.dma_start(s1loc[:], red1[:])
            nc.gpsimd.collective_compute(
                "AllReduce", OP.max, replica_groups=RG,
                ins=[s1loc[:].opt()], outs=[s1sh[:].opt()])
            sam1 = singles.tile([128, 1], FP32, tag="sam1", name="sam1")
            ap1 = s1sh[:]
            nc.gpsimd.dma_start(sam1[:], bass.AP(
                tensor=ap1.tensor, offset=ap1.offset,
                ap=[[0, 128]] + list(ap1.ap)[1:]))
            tsw1 = singles.tile([128, 1], FP32, tag="tsw1", name="tsw1")
            nc.vector.tensor_scalar(tsw1[:], sam1[:], inv2688, None, OP.mult)
            dw1 = singles.tile([128, 1], FP32, tag="dw1", name="dw1")
            nc.vector.tensor_scalar(dw1[:], tsw1[:], 6.0, None, OP.mult)
            rdw1 = singles.tile([128, 1], FP32, tag="rdw1", name="rdw1")
            nc.vector.reciprocal(rdw1[:], dw1[:])
            s_h = singles.tile([128, 1], FP32, tag="s_h", name="s_h")
            nc.vector.tensor_scalar(s_h[:], tsw1[:], float(isc), None, OP.mult)

            # ---- quantize w1 (column-chunked; w1T transposes chase) ----
            for c in range(KC):
                csl = slice(c * 1024, (c + 1) * 1024)
                for j in range(HSH // 128):
                    wt = p0src.tile([128, 1024], FP32, tag="src", name="w1t")
                    nc.scalar.dma_start(
                        wt[:], w1_sh[j * 128:(j + 1) * 128, csl])
                    wo = pb.tile([128, 1024], BF16, tag="qout", name="w1o")
                    _emit_quant(nc, mybir, pf, pb, pn, wt[:], wo[:],
                                rdw1[:], tsw1[:], 1024, qeng())
                    nc.sync.dma_start(w1q[j * 128:(j + 1) * 128, csl], wo[:])
                for j in range(8):
                    kt = c * 8 + j
                    nc.sync.dma_start(w1T[:, kt, :],
                                      w1q[:, kt * 128:(kt + 1) * 128],
                                      transpose=True)

            # ============ Phase 1 main loop ============
            acc2 = singles.tile([128, 1], FP32, tag="acc2", name="acc2")
            tsw2 = singles.tile([128, 1], FP32, tag="tsw2", name="tsw2")
            rdw2 = singles.tile([128, 1], FP32, tag="rdw2", name="rdw2")
            s_o = singles.tile([128, 1], FP32, tag="s_o", name="s_o")

            for t in range(NBT):
                g0 = t * 128
                xb = xb_pool.tile([128, NK1, 128], BF16, tag="xb", name="xb")
                for c in range(KC):
                    nc.sync.dma_start(
                        xb[:, c * 8:(c + 1) * 8, :],
                        xqT_fs[c][g0:g0 + 128, :]
                        .rearrange("p (k b) -> p k b", b=128))
                pss = [ps1.tile([128, 512], FP32, name="ps", tag="ps")
                       for _ in range(4)]
                for kt in range(NK1):
                    for n in range(4):
                        nc.tensor.matmul(
                            pss[n][:], lhsT=xb[:, kt, :],
                            rhs=w1T[:, kt, n * 512:(n + 1) * 512],
                            start=(kt == 0), stop=(kt == NK1 - 1))
                for half in range(2):
                    g = pf.tile([128, 1024], FP32, tag="q_g", name="q_g")
                    for n2 in range(2):
                        nc.scalar.activation(
                            g[:, n2 * 512:(n2 + 1) * 512],
                            pss[half * 2 + n2][:], AF.Gelu, scale=s_h[:])
                    ho = pb.tile([128, 1024], BF16, tag="qout", name="ho")
                    _emit_quant(nc, mybir, pf, pb, pn, g[:], ho[:],
                                c1h, float(hsc), 1024, qeng())
                    nc.sync.dma_start(
                        hq[g0:g0 + 128, half * 1024:(half + 1) * 1024], ho[:])

                # ---- w2 amax / AllReduce / quant interleave ----
                if t < 16:
                    for u in range(2):
                        wi = 2 * t + u
                        for cc in range(2):
                            wt3 = p0src.tile([128, 1024], FP32, tag="wsrc",
                                             name="w2a", bufs=2)
                            nc.scalar.dma_start(
                                wt3[:], w2_sh[wi * 128:(wi + 1) * 128,
                                              cc * 1024:(cc + 1) * 1024])
                            am2 = pn.tile([128, 1], FP32, tag="am_w",
                                          name="am_w2")
                            nc.vector.tensor_reduce(
                                am2[:], wt3[:], axis=mybir.AxisListType.X,
                                op=OP.max, apply_absolute_value=True)
                            if wi == 0 and cc == 0:
                                nc.vector.tensor_copy(acc2[:], am2[:])
                            else:
                                nc.vector.tensor_tensor(acc2[:], acc2[:],
                                                        am2[:], OP.max)
                elif t == 16:
                    nc.sync.dma_start(amax_stage[:, 1:2], acc2[:])
                    rowv2 = singles.tile([1, 128], FP32, tag="rowv2",
                                         name="rowv2")
                    nc.sync.dma_start(
                        rowv2[:],
                        amax_stage[:, 1:2].rearrange("p c -> (p c)").unsqueeze(0))
                    red2 = singles.tile([1, 1], FP32, tag="red2", name="red2")
                    nc.vector.tensor_reduce(red2[:], rowv2[:],
                                            axis=mybir.AxisListType.X, op=OP.max)
                    nc.sync.dma_start(s2loc[:], red2[:])
                    nc.gpsimd.collective_compute(
                        "AllReduce", OP.max, replica_groups=RG,
                        ins=[s2loc[:].opt()], outs=[s2sh[:].opt()])
                    sam2 = singles.tile([128, 1], FP32, tag="sam2", name="sam2")
                    ap2 = s2sh[:]
                    nc.gpsimd.dma_start(sam2[:], bass.AP(
                        tensor=ap2.tensor, offset=ap2.offset,
                        ap=[[0, 128]] + list(ap2.ap)[1:]))
                    nc.vector.tensor_scalar(tsw2[:], sam2[:], inv2688, None,
                                            OP.mult)
                    dw2 = singles.tile([128, 1], FP32, tag="dw2", name="dw2")
                    nc.vector.tensor_scalar(dw2[:], tsw2[:], 6.0, None, OP.mult)
                    nc.vector.reciprocal(rdw2[:], dw2[:])
                    nc.vector.tensor_scalar(s_o[:], tsw2[:], float(hsc), None,
                                            OP.mult)
                elif 18 <= t < 50:
                    wi = t - 18
                    for cc in range(2):
                        sl = slice(cc * 1024, (cc + 1) * 1024)
                        wt2 = p0src.tile([128, 1024], FP32, tag="src",
                                         name="w2t")
                        nc.scalar.dma_start(
                            wt2[:], w2_sh[wi * 128:(wi + 1) * 128, sl])
                        wo2 = pb.tile([128, 1024], BF16, tag="qout", name="w2o")
                        _emit_quant(nc, mybir, pf, pb, pn, wt2[:], wo2[:],
                                    rdw2[:], tsw2[:], 1024, qeng())
                        nc.sync.dma_start(
                            w2q[wi * 128:(wi + 1) * 128, sl], wo2[:])

        # ============ Phase 2 ============
        w1T_cm.__exit__(None, None, None)
        with tc.tile_pool(name="w2T", bufs=1) as w2T_pool, \
             tc.tile_pool(name="hT", bufs=2) as hT_pool, \
             tc.tile_pool(name="osb", bufs=4) as osb, \
             tc.tile_pool(name="ocvt", bufs=2) as ocvt, \
             tc.tile_pool(name="ps2", bufs=8, space="PSUM") as ps2:
            w2T = w2T_pool.tile([128, NK2, D_OUT], BF16, tag="w2T", name="w2T")
            for kt in range(NK2):
                for half in range(2):
                    nc.sync.dma_start(
                        w2T[:, kt, half * 2048:(half + 1) * 2048],
                        w2q[half * 2048:(half + 1) * 2048,
                            kt * 128:(kt + 1) * 128],
                        transpose=True)
            for sb in range(NSB):
                r0 = sb * SBR
                hT = hT_pool.tile([128, NK2, SBR], BF16, tag="hT", name="hT")
                for kt in range(NK2):
                    nc.sync.dma_start(hT[:, kt, :],
                                      hq[r0:r0 + SBR, kt * 128:(kt + 1) * 128],
                                      transpose=True)
                if debug and sb == 0:
                    nc.sync.dma_start(
                        dbg_w2T, w2T[:].rearrange("p k d -> p (k d)"))
                    nc.sync.dma_start(
                        dbg_hT0, hT[:].rearrange("p k d -> p (k d)"))
                for b in range(SBR // 128):
                    row = r0 + b * 128
                    c = row // RSROWS
                    crow = row % RSROWS
                    for half in range(2):
                        pss = [ps2.tile([128, 512], FP32, name="ps2", tag="ps2")
                               for _ in range(4)]
                        for kt in range(NK2):
                            for n in range(4):
                                nc.tensor.matmul(
                                    pss[n][:],
                                    lhsT=hT[:, kt, b * 128:(b + 1) * 128],
                                    rhs=w2T[:, kt,
                                            half * 2048 + n * 512:
                                            half * 2048 + (n + 1) * 512],
                                    start=(kt == 0), stop=(kt == NK2 - 1))
                        ot = osb.tile([128, 2048], BF16, tag="ot", name="ot")
                        for n in range(4):
                            nc.scalar.activation(ot[:, n * 512:(n + 1) * 512],
                                                 pss[n][:], AF.Copy,
                                                 scale=s_o[:])
                        nc.sync.dma_start(
                            parts[c][crow:crow + 128,
                                     half * 2048:(half + 1) * 2048], ot[:])
                if sb % 2 == 1:
                    c = sb // 2
                    nc.gpsimd.collective_compute(
                        "ReduceScatter", OP.add, replica_groups=RG,
                        ins=[parts[c][:].opt()], outs=[rsouts[c][:].opt()])
                    for cc in range(2):
                        osl = slice(cc * 2048, (cc + 1) * 2048)
                        ob = ocvt.tile([128, 2048], BF16, tag="ob", name="ob")
                        nc.scalar.dma_start(ob[:], rsouts[c][:, osl])
                        of = ocvt.tile([128, 2048], FP32, tag="of", name="of")
                        nc.vector.tensor_copy(of[:], ob[:])
                        nc.sync.dma_start(out_sh[c * 128:(c + 1) * 128, osl],
                                          of[:])
        if debug:
            nc.sync.dma_start(dbg_xq, xq_loc[:])
            nc.sync.dma_start(dbg_w1q, w1q[:])
            nc.sync.dma_start(dbg_w2q, w2q[:])
            nc.sync.dma_start(dbg_hq, hq[:])
            nc.sync.dma_start(dbg_xf0, xqT_fs[0][:])
            nc.sync.dma_start(dbg_part0, parts[0][:])
            nc.sync.dma_start(dbg_rsout0, rsouts[0][:])
    nc.compile()
    return nc


def _get_built(isc, hsc):
    key = (float(isc), float(hsc), os.environ.get("KQ_DEBUG", "0"),
           os.environ.get("KQ_NOGPS", "0"))
    if key not in _BUILT:
        _BUILT[key] = _build(float(isc), float(hsc))
    return _BUILT[key]


def run(x, w1, w2, input_scale, hidden_scale, trace=False):
    from concourse import bass_utils
    isc = float(np.asarray(input_scale).reshape(-1)[0])
    hsc = float(np.asarray(hidden_scale).reshape(-1)[0])
    nc = _get_built(isc, hsc)
    x = np.ascontiguousarray(x, dtype=np.float32)
    w1 = np.ascontiguousarray(w1, dtype=np.float32)
    w2 = np.ascontiguousarray(w2, dtype=np.float32)
    in_maps = []
    for c in range(NCORES):
        in_maps.append({
            "x_sh": x[c * BSH:(c + 1) * BSH, :],
            "w1_sh": np.ascontiguousarray(w1[c * HSH:(c + 1) * HSH, :]),
            "w2_sh": np.ascontiguousarray(w2[:, c * HSH:(c + 1) * HSH]),
        })
    res = bass_utils.run_bass_kernel_spmd(
        nc, in_maps, core_ids=list(range(NCORES)), trace=trace)
    out = np.empty((B, D_OUT), dtype=np.float32)
    for r in range(NCORES):
        o = res.results[r]["out_sh"]
        for c in range(RSCH):
            out[c * RSROWS + r * 128:c * RSROWS + (r + 1) * 128, :] = \
                o[c * 128:(c + 1) * 128, :]
    return out, res


def kernel(x, w1, w2, input_scale, hidden_scale):
    out, _ = run(x, w1, w2, input_scale, hidden_scale, trace=False)
    return out
